# revision 3
# baseline (speedup 1.0000x reference)
"""GCN 3-layer message-passing kernel for TRN2 (8 NeuronCores, SPMD) — v5: fp8 L1/L2 tables.

v2 -> v3:
  - 4 row-slice AllGathers per layer (one chunk each, <=32767 rows),
    fired at quarter boundaries of the block loop for finer overlap;
    gather bucket q == source slice == SWDGE queue.
  - Exact gather counts via 4 per-queue Pool registers (trailing -1
    padding): ~35% fewer descriptors and gather bytes vs 0-padding.
  - h1T/h2T stored block-major ([nb, 2, 128, 128]) so hT writes and L3
    reads are single contiguous DMAs.
  - DMA dispatch spread across engines: reads on vector, table writes on
    scalar, rest on sync.
"""

import os
import sys

sys.path.insert(0, "/opt/trn_rl_repo")

from contextlib import ExitStack

import numpy as np
import ml_dtypes

import concourse.bass as bass  # noqa: F401
import concourse.mybir as mybir
import concourse.tile as tile
from concourse import bacc
from concourse._compat import cdiv
from concourse.bass_utils import run_bass_kernel_spmd

F32 = mybir.dt.float32
F8 = mybir.dt.float8e4
BF16 = mybir.dt.bfloat16
I16 = mybir.dt.int16
I32 = mybir.dt.int32
AL = mybir.AluOpType
AF = mybir.ActivationFunctionType

NC = 8
P = 128
SLICE_BLOCKS = [25, 25, 24, 24]  # 98 blocks

LAST_EXEC_NS = None
LAST_SCOPES = None


def _cdiv_np(a, b):
    return (a + b - 1) // b


def _group_cumcount(grp: np.ndarray) -> np.ndarray:
    n = len(grp)
    if n == 0:
        return np.zeros(0, dtype=np.int64)
    is_new = np.ones(n, dtype=bool)
    is_new[1:] = grp[1:] != grp[:-1]
    idx = np.arange(n)
    start = np.maximum.accumulate(np.where(is_new, idx, 0))
    return idx - start


def _slice_geometry(S):
    ends = np.cumsum(SLICE_BLOCKS) * P  # block-row ends per slice
    starts = np.concatenate([[0], ends[:-1]])
    ends = np.minimum(ends, S)
    rows = ends - starts  # rows per core per slice
    return starts, ends, rows


def _preprocess(edge_index: np.ndarray, n_nodes: int):
    N = n_nodes
    S = N // NC
    n_blocks = cdiv(S, P)
    assert sum(SLICE_BLOCKS) == n_blocks
    sstart, send, srows = _slice_geometry(S)
    assert all(r * NC <= 32767 for r in srows)

    src = np.concatenate([edge_index[0], np.arange(N, dtype=np.int64)])
    dst = np.concatenate([edge_index[1], np.arange(N, dtype=np.int64)])
    deg = np.bincount(dst, minlength=N).astype(np.float64)
    dis = (1.0 / np.sqrt(deg)).astype(np.float32)

    core = dst // S
    block = (dst % S) // P
    t_local = (dst % S) % P

    sc = src // S
    sloc = src % S
    q = np.searchsorted(send, sloc, side="right")
    src_local = sc * srows[q] + (sloc - sstart[q])

    nq = 4
    counts = np.zeros((NC, n_blocks, nq), dtype=np.int64)
    np.add.at(counts, (core, block, q), 1)
    slots = np.maximum(1, _cdiv_np(counts.max(axis=0), P))  # [n_blocks, nq]

    K_total = int(slots.sum())
    IW = K_total * 8

    ix_off = np.zeros((n_blocks, nq), dtype=np.int64)
    acc = 0
    for b in range(n_blocks):
        for qq in range(nq):
            ix_off[b, qq] = acc
            acc += int(slots[b, qq])
    assert acc == K_total

    order = np.lexsort((src, q, block, core))
    so_sl = src_local[order]
    so_tl = t_local[order]
    so_core = core[order]
    so_block = block[order]
    so_q = q[order]

    per_core = []
    for c in range(NC):
        m = so_core == c
        cb, cq = so_block[m], so_q[m]
        csl, ctl = so_sl[m], so_tl[m]
        grp = cb * nq + cq
        pos = _group_cumcount(grp)

        idx16 = np.full((16, IW), -1, dtype=np.int16)
        tn = np.full((P, K_total), -1.0, dtype=np.float32)

        col = ix_off[cb, cq] * 8 + pos // 16
        idx16[pos % 16, col] = csl.astype(np.int16)
        tn[pos % P, ix_off[cb, cq] + pos // P] = ctl.astype(np.float32)

        cnt = counts[c]  # [n_blocks, nq]
        empty = cnt == 0
        if empty.any():
            for b, qq in zip(*np.nonzero(empty)):
                idx16[0, ix_off[b, qq] * 8] = 0
        cnts = np.maximum(cnt, 1).astype(np.int32).reshape(-1)

        per_core.append(
            {
                "idx16": np.tile(idx16, (8, 1)),
                "tn": tn.astype(ml_dtypes.bfloat16),
                "cnts": cnts[None, :],
            }
        )

    return {
        "slots": slots,
        "K_total": K_total,
        "IW": IW,
        "ix_off": ix_off,
        "n_blocks": n_blocks,
        "srows": srows,
        "per_core": per_core,
        "dis": dis,
    }


def _build_program(meta, n_nodes, fin, fh, fout):
    N = n_nodes
    S = N // NC
    nb = meta["n_blocks"]
    srows = meta["srows"]
    slots = meta["slots"]
    K_total = meta["K_total"]
    IW = meta["IW"]
    ix_off = meta["ix_off"]
    fo_pad = 128
    sstart = np.concatenate([[0], np.cumsum(SLICE_BLOCKS)[:-1]]) * P
    s_end_blocks = np.cumsum(SLICE_BLOCKS) - 1  # fire ag after these blocks

    nc = bacc.Bacc(num_swdge_queues=4)

    xT = nc.dram_tensor("xT", [fin, S], BF16, kind="ExternalInput")
    W1 = nc.dram_tensor("W1", [fin, fh], BF16, kind="ExternalInput")
    W2 = nc.dram_tensor("W2", [fh, fh], BF16, kind="ExternalInput")
    W3 = nc.dram_tensor("W3", [fh, fout], BF16, kind="ExternalInput")
    LW = nc.dram_tensor("LW", [2 * fh + fout, fout], BF16, kind="ExternalInput")
    idx16 = nc.dram_tensor("idx16", [P, IW], I16, kind="ExternalInput")
    tn_in = nc.dram_tensor("tn", [P, K_total], BF16, kind="ExternalInput")
    iota_in = nc.dram_tensor("iota", [P, P], BF16, kind="ExternalInput")
    ident_in = nc.dram_tensor("ident", [P, P], BF16, kind="ExternalInput")
    dis_in = nc.dram_tensor("disblk", [P, nb], F32, kind="ExternalInput")
    bd1_in = nc.dram_tensor("bd1", [nb * P, fh], BF16, kind="ExternalInput")
    bd2_in = nc.dram_tensor("bd2", [nb * P, fh], BF16, kind="ExternalInput")
    bd3_in = nc.dram_tensor("bd3", [nb * P, fout], BF16, kind="ExternalInput")
    lbbc_in = nc.dram_tensor("lbbc", [P, fout], BF16, kind="ExternalInput")
    cnts_in = nc.dram_tensor("cnts", [1, nb * 4], I32, kind="ExternalInput")
    out_sh = nc.dram_tensor("out_sh", [S, fout], F32, kind="ExternalOutput")

    t1_sh = nc.dram_tensor("t1_sh", [S, fh], F8)
    t2_sh = nc.dram_tensor("t2_sh", [S, fh], F8)
    t3_sh = nc.dram_tensor("t3_sh", [S, fo_pad], BF16)
    h1T_sh = nc.dram_tensor("h1T_sh", [nb, 2, P, P], BF16)
    h2T_sh = nc.dram_tensor("h2T_sh", [nb, 2, P, P], BF16)

    tabs = {}
    tab_dt = {1: F8, 2: F8, 3: BF16}
    for li, fel in ((1, fh), (2, fh), (3, fo_pad)):
        tabs[li] = tuple(
            nc.dram_tensor(f"t{li}S{s}", [int(srows[s]) * NC, fel], tab_dt[li],
                           addr_space="Shared")
            for s in range(4)
        )
    t_shs = {1: t1_sh, 2: t2_sh, 3: t3_sh}

    rg = [list(range(NC))]

    def used_rows(b):
        return min(P, S - b * P)

    def fire_ag(li, s):
        t_sh = t_shs[li]
        r0, r1 = int(sstart[s]), int(sstart[s]) + int(srows[s])
        sc = nc.enter_named_scope(f"ag{li}S{s}", False)
        nc.gpsimd.collective_compute(
            "AllGather", AL.bypass, ins=[t_sh[r0:r1, :]], outs=[tabs[li][s][:, :]],
            replica_groups=rg,
        )
        nc.leave_named_scope(f"ag{li}S{s}", sc[0], False)

    with tile.TileContext(nc) as tc:
        with (
            tc.tile_pool(name="const", bufs=1) as cpool,
            tc.tile_pool(name="sb", bufs=3) as pool,
            tc.tile_pool(name="gath", bufs=3) as gpool,
            tc.tile_pool(name="st", bufs=3) as stpool,
            tc.tile_pool(name="ps", bufs=2, space="PSUM") as psp,
            ExitStack() as regs_ctx,
        ):
            cnt_regs = [
                regs_ctx.enter_context(nc.gpsimd.register(f"cnt{q}"))
                for q in range(4)
            ]
            iota_t = cpool.tile([P, P], BF16)
            nc.sync.dma_start(out=iota_t[:], in_=iota_in[:, :])
            ident_t = cpool.tile([P, P], BF16)
            nc.sync.dma_start(out=ident_t[:], in_=ident_in[:, :])
            w1_t = cpool.tile([P, 2, fh], BF16)
            nc.sync.dma_start(out=w1_t[:], in_=W1[:, :].rearrange("(c k) f -> k c f", k=P))
            w2_t = cpool.tile([P, 2, fh], BF16)
            nc.sync.dma_start(out=w2_t[:], in_=W2[:, :].rearrange("(c k) f -> k c f", k=P))
            w3_t = cpool.tile([P, 2, fout], BF16)
            nc.sync.dma_start(out=w3_t[:], in_=W3[:, :].rearrange("(c k) f -> k c f", k=P))
            lw12_t = cpool.tile([P, 4, fout], BF16)
            nc.sync.dma_start(
                out=lw12_t[:], in_=LW[: 4 * P, :].rearrange("(c k) f -> k c f", k=P)
            )
            lw3_t = cpool.tile([fout, fout], BF16)
            nc.sync.dma_start(out=lw3_t[:], in_=LW[4 * P :, :])
            lbbc = cpool.tile([P, fout], BF16)
            nc.sync.dma_start(out=lbbc[:], in_=lbbc_in[:, :])
            dis_t = cpool.tile([P, nb], F32)
            nc.sync.dma_start(out=dis_t[:], in_=dis_in[:, :])
            idx_t = cpool.tile([P, IW], I16)
            nc.sync.dma_start(out=idx_t[:], in_=idx16[:, :])
            tn_t = cpool.tile([P, K_total], BF16)
            nc.sync.dma_start(out=tn_t[:], in_=tn_in[:, :])
            cnts_t = cpool.tile([1, nb * 4], I32)
            nc.sync.dma_start(out=cnts_t[:], in_=cnts_in[:, :])

            for _w in range(3):  # gath pool bufs
                for qq in range(4):
                    warm = gpool.tile([P, 6, fh], F8, tag=f"dst{qq}")
                    nc.vector.memset(warm[:], 0.0)
            for _w in range(3):
                for qq in range(4):
                    warm = gpool.tile([P, 6, 128], BF16, tag=f"dst{qq}")
                    nc.vector.memset(warm[:], 0.0)

            sc_T = nc.enter_named_scope("phaseT", False)
            for b in range(nb):
                u = used_rows(b)
                ps1 = psp.tile([P, fh], F32, tag="ps2")
                for cc in range(2):
                    xt = pool.tile([P, P], BF16, tag="xt")
                    nc.sync.dma_start(
                        out=xt[:, :u], in_=xT[cc * P : (cc + 1) * P, b * P : b * P + u]
                    )
                    nc.tensor.matmul(
                        ps1[:u, :],
                        lhsT=xt[:, :u],
                        rhs=w1_t[:, cc, :],
                        start=(cc == 0),
                        stop=(cc == 1),
                    )
                ev = pool.tile([P, fh], F8, tag="evq")
                nc.scalar.activation(
                    ev[:u, :], ps1[:u, :], AF.Copy, scale=dis_t[:u, b : b + 1]
                )
                nc.scalar.dma_start(out=t1_sh[b * P : b * P + u, :], in_=ev[:u, :])
                for s in range(4):
                    if b == int(s_end_blocks[s]):
                        fire_ag(1, s)
            nc.leave_named_scope("phaseT", sc_T[0], False)

            def layer(li, felem):
                fagg = fh if li < 3 else fout
                for b in range(nb):
                    u = used_rows(b)
                    kb = int(slots[b].sum())
                    soff = int(ix_off[b, 0])
                    dsts = []
                    for qq in range(4):
                        sl = int(slots[b, qq])
                        tab = tabs[li][qq]
                        rows = int(srows[qq]) * NC
                        g_dt = F8 if li < 3 else BF16
                        dst = gpool.tile([P, 6, felem], g_dt, tag=f"dst{qq}")
                        io = int(ix_off[b, qq]) * 8
                        nc.gpsimd.reg_load(
                            cnt_regs[qq], cnts_t[0:1, b * 4 + qq : b * 4 + qq + 1]
                        )
                        nc.gpsimd.dma_gather(
                            dst[:, :sl, :],
                            tab[0:rows, :],
                            idx_t[:, io : io + sl * 8],
                            sl * P,
                            cnt_regs[qq],
                            felem,
                            single_packet=False,
                            queue_num=qq,
                        )
                        dsts.append(dst)

                    st_t = stpool.tile([P, kb, P], F8 if li < 3 else BF16, tag="st")
                    in0 = iota_t[:, :].unsqueeze(1).broadcast_to([P, kb, P])
                    in1 = (
                        tn_t[:, soff : soff + kb]
                        .unsqueeze(2)
                        .broadcast_to([P, kb, P])
                    )
                    nc.vector.tensor_tensor(
                        out=st_t[:, :, :], in0=in0, in1=in1, op=AL.is_equal
                    )

                    bdt = pool.tile([P, fagg], BF16, tag="bdt")
                    bd_in = (bd1_in, bd2_in, bd3_in)[li - 1]
                    nc.sync.dma_start(
                        out=bdt[:, :], in_=bd_in[b * P : (b + 1) * P, :]
                    )

                    psa = psp.tile([P, fagg], F32, tag="psa")
                    s = 0
                    for qq in range(4):
                        sl = int(slots[b, qq])
                        for j in range(sl):
                            nc.tensor.matmul(
                                psa[:],
                                lhsT=st_t[:, s, :],
                                rhs=dsts[qq][:, j, :fagg],
                                start=(s == 0),
                                stop=False,
                            )
                            s += 1
                    nc.tensor.matmul(
                        psa[:], lhsT=ident_t[:], rhs=bdt[:],
                        start=False, stop=True,
                    )
                    h_sb = pool.tile([P, fagg], BF16, tag="h_sb")
                    nc.scalar.activation(
                        h_sb[:u, :], psa[:u, :], AF.Relu, scale=dis_t[:u, b : b + 1]
                    )

                    if li < 3:
                        wnext = w2_t if li == 1 else w3_t
                        fnext = fh if li == 1 else fout
                        hT_sh_ = h1T_sh if li == 1 else h2T_sh
                        tnext = t2_sh if li == 1 else t3_sh
                        ps2 = psp.tile([P, fnext], F32, tag="ps2")
                        hT2 = pool.tile([P, 2, P], BF16, tag="hT2")
                        for cc in range(2):
                            pst = psp.tile([P, P], BF16, tag=f"pst{cc}")
                            nc.tensor.transpose(
                                pst[:], h_sb[:, cc * P : (cc + 1) * P], ident_t[:]
                            )
                            nc.vector.tensor_copy(hT2[:, cc, :], pst[:])
                            nc.tensor.matmul(
                                ps2[:u, :],
                                lhsT=hT2[:, cc, :u],
                                rhs=wnext[:, cc, :fnext],
                                start=(cc == 0),
                                stop=(cc == 1),
                            )
                        nc.scalar.dma_start(
                            out=hT_sh_[b, :, :, :u].rearrange("c f n -> f c n"),
                            in_=hT2[:, :, :u],
                        )
                        ev2 = pool.tile([P, fnext], F8 if li == 1 else BF16, tag="ev")
                        nc.scalar.activation(
                            ev2[:u, :fnext], ps2[:u, :], AF.Copy,
                            scale=dis_t[:u, b : b + 1],
                        )
                        nc.scalar.dma_start(
                            out=tnext[b * P : b * P + u, :fnext], in_=ev2[:u, :fnext]
                        )
                        for s4 in range(4):
                            if b == int(s_end_blocks[s4]):
                                fire_ag(li + 1, s4)
                    else:
                        ps3t = psp.tile([P, P], BF16, tag="pst0")
                        nc.tensor.transpose(ps3t[:fout, :], h_sb[:, :fout], ident_t[:])
                        h3T = pool.tile([fout, P], BF16, tag="hT0")
                        nc.vector.tensor_copy(h3T[:], ps3t[:fout, :])
                        pso = psp.tile([P, fout], F32, tag="ps2")
                        r1 = pool.tile([P, 2, P], BF16, tag="rl1")
                        nc.scalar.dma_start(
                            out=r1[:, :, :u],
                            in_=h1T_sh[b, :, :, :u].rearrange("c f n -> f c n"),
                        )
                        r2 = pool.tile([P, 2, P], BF16, tag="rl2")
                        nc.scalar.dma_start(
                            out=r2[:, :, :u],
                            in_=h2T_sh[b, :, :, :u].rearrange("c f n -> f c n"),
                        )
                        for cc in range(2):
                            nc.tensor.matmul(
                                pso[:u, :], lhsT=r1[:, cc, :u], rhs=lw12_t[:, cc, :],
                                start=(cc == 0), stop=False,
                            )
                        for cc in range(2):
                            nc.tensor.matmul(
                                pso[:u, :], lhsT=r2[:, cc, :u], rhs=lw12_t[:, 2 + cc, :],
                                start=False, stop=False,
                            )
                        nc.tensor.matmul(
                            pso[:u, :], lhsT=h3T[:, :u], rhs=lw3_t[:, :],
                            start=False, stop=False,
                        )
                        nc.tensor.matmul(
                            pso[:u, :], lhsT=ident_t[:, :u], rhs=lbbc[:, :],
                            start=False, stop=True,
                        )
                        m_t = pool.tile([P, 1], F32, tag="m_t")
                        nc.vector.tensor_reduce(
                            m_t[:u, :], pso[:u, :], mybir.AxisListType.X, AL.max
                        )
                        nm_t = pool.tile([P, 1], F32, tag="nm_t")
                        nc.vector.tensor_scalar(
                            out=nm_t[:u, :], in0=m_t[:u, :], scalar1=-1.0,
                            scalar2=None, op0=AL.mult,
                        )
                        e_t = pool.tile([P, fout], F32, tag="e_t")
                        ssum = pool.tile([P, 1], F32, tag="ssum")
                        nc.scalar.activation(
                            e_t[:u, :], pso[:u, :], AF.Exp,
                            bias=nm_t[:u, :1], accum_out=ssum[:u, :1],
                        )
                        ls_t = pool.tile([P, 1], F32, tag="ls_t")
                        nc.scalar.activation(ls_t[:u, :], ssum[:u, :], AF.Ln)
                        mls = pool.tile([P, 1], F32, tag="mls")
                        nc.vector.tensor_tensor(
                            out=mls[:u, :], in0=m_t[:u, :], in1=ls_t[:u, :], op=AL.add
                        )
                        z_t = pool.tile([P, fout], F32, tag="z_t")
                        nc.vector.tensor_scalar(
                            out=z_t[:u, :], in0=pso[:u, :], scalar1=mls[:u, :1],
                            scalar2=None, op0=AL.subtract,
                        )
                        nc.sync.dma_start(
                            out=out_sh[b * P : b * P + u, :], in_=z_t[:u, :]
                        )

            sc = nc.enter_named_scope("L1", False)
            layer(1, fh)
            nc.leave_named_scope("L1", sc[0], False)
            sc = nc.enter_named_scope("L2", False)
            layer(2, fh)
            nc.leave_named_scope("L2", sc[0], False)
            sc = nc.enter_named_scope("L3", False)
            layer(3, fo_pad)
            nc.leave_named_scope("L3", sc[0], False)

    nc.finalize()
    return nc


def kernel(x, edge_index, W1, b1, W2, b2, W3, b3, lin_w, lin_b):
    global LAST_EXEC_NS, LAST_SCOPES
    x = np.asarray(x)
    N = x.shape[0]
    S = N // NC
    fin, fh, fout = W1.shape[0], W2.shape[0], W3.shape[1]

    meta = _preprocess(np.asarray(edge_index, dtype=np.int64), N)
    nc = _build_program(meta, N, fin, fh, fout)

    dis = meta["dis"]
    nb = meta["n_blocks"]

    iota = np.tile(np.arange(P, dtype=np.float32), (P, 1)).astype(ml_dtypes.bfloat16)
    ident = np.eye(P, dtype=np.float32).astype(ml_dtypes.bfloat16)
    lbbc = np.tile(np.asarray(lin_b, np.float32), (P, 1)).astype(ml_dtypes.bfloat16)

    in_maps = []
    for c in range(NC):
        xs = np.asarray(x[c * S : (c + 1) * S], np.float32)
        dc = dis[c * S : (c + 1) * S]
        dis_blk = np.ones((P, nb), dtype=np.float32)
        for b in range(nb):
            u = min(P, S - b * P)
            dis_blk[:u, b] = dc[b * P : b * P + u]
        dpad = np.ones(nb * P, dtype=np.float32)
        dpad[:S] = dc
        bd1 = (np.asarray(b1, np.float32)[None, :] / dpad[:, None]).astype(
            ml_dtypes.bfloat16
        )
        bd2 = (np.asarray(b2, np.float32)[None, :] / dpad[:, None]).astype(
            ml_dtypes.bfloat16
        )
        bd3 = (np.asarray(b3, np.float32)[None, :] / dpad[:, None]).astype(
            ml_dtypes.bfloat16
        )
        in_maps.append(
            {
                "xT": np.ascontiguousarray(xs.T).astype(ml_dtypes.bfloat16),
                "W1": np.asarray(W1, np.float32).astype(ml_dtypes.bfloat16),
                "W2": np.asarray(W2, np.float32).astype(ml_dtypes.bfloat16),
                "W3": np.asarray(W3, np.float32).astype(ml_dtypes.bfloat16),
                "LW": np.asarray(lin_w, np.float32).astype(ml_dtypes.bfloat16),
                "idx16": meta["per_core"][c]["idx16"],
                "tn": meta["per_core"][c]["tn"],
                "cnts": meta["per_core"][c]["cnts"],
                "iota": iota,
                "ident": ident,
                "disblk": dis_blk,
                "bd1": bd1,
                "bd2": bd2,
                "bd3": bd3,
                "lbbc": lbbc,
            }
        )
    trace = bool(os.environ.get("GCN_TRACE"))
    res = run_bass_kernel_spmd(nc, in_maps, list(range(NC)), trace=trace)
    LAST_EXEC_NS = res.exec_time_ns
    LAST_SCOPES = res.per_core_scope_times
    out = np.concatenate([res.results[c]["out_sh"] for c in range(NC)], axis=0)
    return out.astype(np.float32)
